# revision 19
# baseline (speedup 1.0000x reference)
"""Trainium2 Bass kernel for nn_DihedralAngleLayer.

Input:  x [2_000_000, 42] f32 (14 atoms x 3 coords per row),
        mask_matrix [4, 14] f32 one-hot carbon selector.
Output: dihedral angle per row, [2_000_000] f32.

Data-parallel across 8 NeuronCores: rows are padded to 8*250_112 and split
evenly. Each core owns rows in global partition-major order: partition p
handles rows [p*Q, (p+1)*Q), Q = rows/128. Per tile (G columns of every
partition) the Vector engine computes

    a = c0-c1, b = c2-c1, d = c3-c2
    na = a x b, nb = d x b
    det = na.d, xx = na.nb, q = b.b      (one 9-read segmented reduce
                                          straight into full planes)
    yy = sqrt(q)*det

The x/y duplicates the shifted-AP cross reads need are written by ScalarE
Copy (only 2 dup elems per vector are ever read); q = b*b runs on ScalarE
Square. The atan2 tail uses the full-range ACT Arctan table (verified
5e-7 max err over all f32 incl +-inf): t = arctan(yy/|xx|) plus a 3-op
quadrant fix from e2=sign(xx), sy=sign(yy).

Because every engine executes its stream in order, tile work is
software-pipelined at emission: iteration i emits DMA(i) + V-subs(i),
stage2(i-1) [crosses/products/reduce], stage3(i-2) [sqrt+yy], tails, and
only then tile i's ScalarE copy/square, so the DVE never sits in a
ScalarE round-trip (the framework's cross-engine waits are conservative
in emission order). x prefetch triggers ride GpSimd, y writebacks ride
ScalarE, so prefetch is never blocked behind tail-dependent work.
"""

import numpy as np

import concourse.bacc as bacc
import concourse.bass as bass
import concourse.mybir as mybir
from concourse.bass_utils import run_bass_kernel_spmd
from concourse.tile import TileContext

AF = mybir.ActivationFunctionType
OP = mybir.AluOpType
F32 = mybir.dt.float32

PI = float(np.pi)

N_CORES = 8
G_TILE = 224
# first tiles smaller so DVE starts sooner (smaller first DMA)
TILES = [48, 96, 144] + [G_TILE] * 7 + [98]   # sum = 1954
CHUNK_AFTER = {3, 7, 10}                 # tile indices closing a tail chunk
Q = sum(TILES)                      # rows per partition
ROWS_PER_CORE = 128 * Q            # 250_112

# row-interleaved scratch layout per row-group (period PER floats):
# a,a'(0..6) b,b'(6..12) d,d'(12..18)
# quad (18..30 transient: P1a P1b P2a P2b); (na,nb) overwrite 18..24
# in place; prods (na.d, na.nb) overwrite 24..30; b.b @30 (stage1)
# reduce reads 24..33 -> PER=33
PER = 33
S_A, S_B, S_D, S_NA, S_NB, S_P, S_Q = 0, 6, 12, 18, 21, 18, 30

# chunk-tail scratch: 6 slots of CS_FD floats each
CS_FD = 896


def _ap(base, off, dims):
    return bass.AP(
        base.tensor, base.offset + off, [list(base.ap[0])] + [list(d) for d in dims]
    )


def _sap(sc, off, dims=()):
    return _ap(sc[:], off, [[PER, sc.shape[1] // PER]] + list(dims))


class _TileCtx:
    """Per-tile handles so stages can be emitted out of order."""

    def __init__(self, nc, xp, scp, x, toff, G):
        self.toff, self.G = toff, G
        self.xt = xp.tile([128, G * 42], F32, tag="x")
        self.sc = scp.tile([128, G * PER], F32, tag="sc")
        nc.gpsimd.dma_start(
            out=self.xt[:],
            in_=x.rearrange("(p q) c -> p q c", p=128)[:, toff : toff + G, :],
        )


def _stage1v(nc, t, c0, c1, c2, c3):
    """subs on DVE."""
    v = nc.vector
    G = t.G
    xa = t.xt[:]

    def xap(off, dims):
        return _ap(xa, off, [[42, G]] + dims)

    def sap(off, dims=()):
        return _sap(t.sc, off, dims)

    # (b, d) x/y = (c2, c3) - (c1, c2); x/y first so the ScalarE dup
    # copy can start while DVE fills in the z comps
    v.tensor_tensor(
        sap(S_B, [[6, 2], [1, 2]]),
        xap(c2, [[12, 2], [1, 2]]),
        xap(c1, [[9, 2], [1, 2]]),
        OP.subtract,
    )
    # a = c0-c1 (all three comps)
    v.tensor_tensor(sap(S_A, [[1, 3]]), xap(c0, [[1, 3]]), xap(c1, [[1, 3]]), OP.subtract)
    # (b, d) z comps, off the copy's critical path
    v.tensor_tensor(
        sap(S_B + 2, [[6, 2], [1, 1]]),
        xap(c2 + 2, [[12, 2], [1, 1]]),
        xap(c1 + 2, [[9, 2], [1, 1]]),
        OP.subtract,
    )


def _stage1s(nc, t):
    """dup copy + b*b on ScalarE.  Emitted AFTER stage2(i-1)/stage3(i-2)
    so the framework's conservative emission-order semaphore for
    P12a(i-1) only covers copy(i-1), not this tile's copy."""
    s = nc.scalar

    def sap(off, dims=()):
        return _sap(t.sc, off, dims)

    # x/y duplicates for the shifted-AP cross reads (z dup is never read)
    s.activation(sap(S_A + 3, [[6, 3], [1, 2]]), sap(S_A, [[6, 3], [1, 2]]), AF.Copy)
    # q = b*b -> S_Q (reduce input, ready long before the reduce)
    s.activation(sap(S_Q, [[1, 3]]), sap(S_B, [[1, 3]]), AF.Square)


def _stage2(nc, t, xyf):
    """crosses, products, fused dot-reduce -> full planes."""
    v = nc.vector
    G, toff = t.G, t.toff

    def sap(off, dims=()):
        return _sap(t.sc, off, dims)

    # a-cross: P1a = a_yzx*b_zxy @18 ; P2a = a_zxy*b_yzx @24
    v.tensor_tensor(
        sap(S_P, [[6, 2], [1, 3]]),
        sap(S_A + 1, [[1, 2], [1, 3]]),
        sap(S_B + 2, [[-1, 2], [1, 3]]),
        OP.mult,
    )
    # d-cross: P1b = d_yzx*b_zxy @21 ; P2b = d_zxy*b_yzx @27
    v.tensor_tensor(
        sap(S_P + 3, [[6, 2], [1, 3]]),
        sap(S_D + 1, [[1, 2], [1, 3]]),
        sap(S_B + 2, [[-1, 2], [1, 3]]),
        OP.mult,
    )
    # (na, nb) = P1 - P2 in place over (P1a, P1b)
    v.tensor_tensor(sap(S_NA, [[1, 6]]), sap(S_P, [[1, 6]]), sap(S_P + 6, [[1, 6]]), OP.subtract)
    # products (na*d, na*nb) -> 24..30 over the dead P2 quad half
    v.tensor_tensor(
        sap(S_P + 6, [[3, 2], [1, 3]]),
        sap(S_NA, [[0, 2], [1, 3]]),
        sap(S_D, [[9, 2], [1, 3]]),
        OP.mult,
    )
    # segmented reduce (det, xx, q) straight into the full planes:
    # det -> xyf[toff], xx -> xyf[Q+toff], q -> xyf[2Q+toff]
    v.reduce_sum(
        _ap(xyf, toff, [[Q, 3], [1, G]]),
        _ap(t.sc[:], S_P + 6, [[3, 3], [PER, G], [1, 3]]),
        axis=mybir.AxisListType.X,
    )


def _emit_tail(nc, csp, outp, y, xyf, toff, FD):
    """Chunk tail: atan2 on [128, FD] contiguous planes via full-range arctan.

    t = arctan(yy/|xx|); theta = e2*t + (pi/2)*(1-e2)*sy with e2=sign(xx),
    sy=sign(yy).  (STT subtract is reversed: out = in1 - (in0 op0 scalar).)
    """
    v, s = nc.vector, nc.scalar

    cs = csp.tile([128, 6 * CS_FD], F32, tag="cs")
    ot = outp.tile([128, CS_FD], F32, tag="o")
    ca = cs[:]

    def cap(k):
        return _ap(ca, k * CS_FD, [[1, FD]])

    # planes: det/yy @ toff, xx @ Q+toff, q @ 2Q+toff
    # slots: 0:ax/w  1:rx/v2  2:rq/t  3:e2  4:sy  5:sq
    s.activation(cap(5), _ap(xyf, 2 * Q + toff, [[1, FD]]), AF.Sqrt)  # sq = sqrt(q)
    v.tensor_tensor(
        _ap(xyf, toff, [[1, FD]]), _ap(xyf, toff, [[1, FD]]), cap(5), OP.mult
    )                                                              # yy = det*sq
    s.activation(cap(0), _ap(xyf, Q + toff, [[1, FD]]), AF.Square)  # xsq = xx^2
    s.activation(cap(1), cap(0), AF.Abs_reciprocal_sqrt)           # rx = 1/|xx|
    s.activation(
        _ap(ca, 3 * CS_FD, [[CS_FD, 2], [1, FD]]),
        _ap(xyf, Q + toff, [[-Q, 2], [1, FD]]),
        AF.Sign,
    )                                                              # e2, sy
    v.tensor_tensor(cap(2), _ap(xyf, toff, [[1, FD]]), cap(1), OP.mult)  # rq = yy*rx
    s.activation(cap(2), cap(2), AF.Arctan)                        # t (in place)
    v.scalar_tensor_tensor(cap(0), cap(3), 1.0, cap(4), OP.subtract, OP.mult)  # w=(e2-1)*sy
    v.tensor_tensor(cap(1), cap(3), cap(2), OP.mult)               # v2 = e2*t
    # out = v2 - w*pi/2  (reversed subtract)
    v.scalar_tensor_tensor(
        _ap(ot[:], 0, [[1, FD]]), cap(0), PI / 2, cap(1), OP.mult, OP.subtract
    )
    # y-write triggers ride ScalarE so the SP stream of x prefetch triggers
    # is never blocked behind a tail-dependent write
    nc.scalar.dma_start(
        out=y.rearrange("(p q) -> p q", p=128)[:, toff : toff + FD],
        in_=_ap(ot[:], 0, [[1, FD]]),
    )


def build_kernel(atoms):
    c0, c1, c2, c3 = (3 * int(a) for a in atoms)
    nc = bacc.Bacc("TRN2", target_bir_lowering=False, debug=False)
    x = nc.dram_tensor("x", [ROWS_PER_CORE, 42], F32, kind="ExternalInput")
    y = nc.dram_tensor("y", [ROWS_PER_CORE], F32, kind="ExternalOutput")
    with TileContext(nc) as tc:
        with (
            tc.tile_pool(name="xp", bufs=2) as xp,
            tc.tile_pool(name="scp", bufs=2) as scp,
            tc.tile_pool(name="xyp", bufs=1) as xyp,
            tc.tile_pool(name="csp", bufs=1) as csp,
            tc.tile_pool(name="outp", bufs=2) as outp,
        ):
            xyf_tile = xyp.tile([128, 3 * Q], F32, tag="xy")
            xyf = xyf_tile[:]

            # tail chunk boundaries in tile indices -> (toff, FD)
            chunks = {}
            toff = 0
            start = 0
            for i, G in enumerate(TILES):
                toff += G
                if i in CHUNK_AFTER or i == len(TILES) - 1:
                    chunks[i] = (start, toff - start)
                    start = toff

            ts = []
            toff = 0

            def pipeline_step(i):
                """emit stage2(i-1) and the tail for a chunk ending at i-1."""
                if 0 <= i - 1 < len(ts):
                    _stage2(nc, ts[i - 1], xyf)
                    if i - 1 in chunks:
                        cstart, cfd = chunks[i - 1]
                        _emit_tail(nc, csp, outp, y, xyf, cstart, cfd)

            for i, G in enumerate(TILES):
                t = _TileCtx(nc, xp, scp, x, toff, G)
                ts.append(t)
                _stage1v(nc, t, c0, c1, c2, c3)
                pipeline_step(i)
                _stage1s(nc, t)
                toff += G
            pipeline_step(len(TILES))
    nc.finalize()
    return nc


_CACHE = {}


def _get_nc(atoms):
    key = tuple(int(a) for a in atoms)
    if key not in _CACHE:
        _CACHE[key] = build_kernel(key)
    return _CACHE[key]


def run(x, atoms=(0, 4, 7, 11), **spmd_kwargs):
    """x: [B, 42] f32. Returns (y [B] f32, BassKernelResults)."""
    x = np.ascontiguousarray(np.asarray(x, dtype=np.float32))
    B = x.shape[0]
    total = N_CORES * ROWS_PER_CORE
    if B < total:
        # pad with replicated leading rows (valid, non-degenerate data)
        x = np.concatenate([x, x[: total - B]], axis=0)
    nc = _get_nc(atoms)
    shards = x.reshape(N_CORES, ROWS_PER_CORE, 42)
    in_maps = [{"x": shards[i]} for i in range(N_CORES)]
    res = run_bass_kernel_spmd(nc, in_maps, core_ids=list(range(N_CORES)), **spmd_kwargs)
    y = np.concatenate([r["y"] for r in res.results])[:B]
    return np.asarray(y, dtype=np.float32), res


def kernel(x, mask_matrix):
    mask = np.asarray(mask_matrix)
    atoms = tuple(int(i) for i in np.argmax(mask, axis=1))
    y, _ = run(x, atoms=atoms)
    return y
